# revision 2
# baseline (speedup 1.0000x reference)
"""Two-layer LSTM (linear cell/output activations) + FC head on 8 NeuronCores.

Strategy (data-parallel over batch, per the sharding hint):
  - B=32 split across 8 cores -> B_local=4 per core; weights replicated.
  - All state kept transposed: h^T/c^T are [H on partitions, (k,b) on free],
    so the per-step recurrence matmul is  z^T = U^T @ h^T  with U as the
    stationary operand and gates landing as [128, 4] column blocks. All
    elementwise gate math then runs on full-partition [128, 8] tiles.
  - Input projections (x@W0+b0, h0@W1+b1) are batched per 64-step chunk so
    their weight loads amortize; only the U-recurrence runs step-by-step.
  - T=2048 processed by a hardware For_i loop over 32 chunks of 64 steps.
  - Matmul operands (weights, x, h) optionally bf16 (fp32 PSUM accumulate):
    fp32 matmuls cost 4 cycles/row on TRN2 and block fast-weight-load;
    bf16 halves the dominant per-step LDWEIGHTS cost. Cell state c and all
    gate math stay fp32.
"""

import os
import numpy as np
from contextlib import ExitStack

os.environ.setdefault("MYCRO_LOCAL_CACHE", "1")

B, T, I, H, O = 32, 2048, 128, 256, 128
NCORES = 8
BL = B // NCORES          # 4 batch elements per core
CHUNK = 64                # timesteps per loop body
G4 = 4 * H                # 1024 gate columns
NM = G4 // 128            # 8 gate chunks of 128
KT = H // 128             # 2 contraction tiles

MM_BF16 = True            # matmul operands in bf16

_cache = {}


def _np_mmdt():
    if MM_BF16:
        import ml_dtypes
        return ml_dtypes.bfloat16
    return np.float32


def _build(tiny=False, mm_bf16=None):
    import concourse.bacc as bacc
    import concourse.bass as bass
    import concourse.tile as tile
    import concourse.mybir as mybir

    if mm_bf16 is None:
        mm_bf16 = MM_BF16
    f32 = mybir.dt.float32
    mdt = mybir.dt.bfloat16 if mm_bf16 else f32
    AF = mybir.ActivationFunctionType
    ALU = mybir.AluOpType

    nc = bacc.Bacc("TRN2", target_bir_lowering=False, debug=False,
                   num_devices=NCORES)

    xprep_d = nc.declare_dram_parameter("xprep", [I, BL, T], mdt, isOutput=False)
    w0_d = nc.declare_dram_parameter("w0", [I, G4], mdt, isOutput=False)
    u0_d = nc.declare_dram_parameter("u0", [H, G4], mdt, isOutput=False)
    w1_d = nc.declare_dram_parameter("w1", [H, G4], mdt, isOutput=False)
    u1_d = nc.declare_dram_parameter("u1", [H, G4], mdt, isOutput=False)
    wfc_d = nc.declare_dram_parameter("wfc", [H, O], mdt, isOutput=False)
    b0t_d = nc.declare_dram_parameter("b0t", [128, NM], f32, isOutput=False)
    b1t_d = nc.declare_dram_parameter("b1t", [128, NM], f32, isOutput=False)
    bfct_d = nc.declare_dram_parameter("bfct", [128, 1], f32, isOutput=False)
    out_d = nc.declare_dram_parameter("outT", [O, BL], f32, isOutput=True)

    with tile.TileContext(nc) as tc, ExitStack() as ctx:
        if tiny:
            pool = ctx.enter_context(tc.tile_pool(name="tp", bufs=1))
            t1 = pool.tile([128, BL], mdt, tag="t1")
            t2 = pool.tile([128, BL], f32, tag="t2")
            nc.sync.dma_start(t1[:, :], xprep_d[:, :, 0])
            nc.vector.tensor_copy(t2[:, :], t1[:, :])
            nc.sync.dma_start(out_d[:, :], t2[:, :])
            nc.compile()
            return nc

        const = ctx.enter_context(tc.tile_pool(name="const", bufs=1))
        work = ctx.enter_context(tc.tile_pool(name="work", bufs=3))
        psum = ctx.enter_context(tc.tile_pool(name="psum", bufs=2, space="PSUM"))

        # Persistent SBUF residents.
        xall = const.tile([128, BL * T], mdt, tag="xall")       # col = b*T + t
        w0 = const.tile([128, G4], mdt, tag="w0")
        u0 = [const.tile([128, G4], mdt, tag=f"u0_{k}", name=f"u0_{k}")
              for k in range(KT)]
        w1 = [const.tile([128, G4], mdt, tag=f"w1_{k}", name=f"w1_{k}")
              for k in range(KT)]
        u1 = [const.tile([128, G4], mdt, tag=f"u1_{k}", name=f"u1_{k}")
              for k in range(KT)]
        wf = [const.tile([128, O], mdt, tag=f"wf_{k}", name=f"wf_{k}")
              for k in range(KT)]
        b0t = const.tile([128, NM], f32, tag="b0t")
        b1t = const.tile([128, NM], f32, tag="b1t")
        bfct = const.tile([128, 1], f32, tag="bfct")
        # chunk buffers: col = k*(BL*CHUNK) + b*CHUNK + t   (h0t)
        #                col = m*(BL*CHUNK) + b*CHUNK + t   (xw0t/xw1t)
        h0t = const.tile([128, KT * BL * CHUNK], mdt, tag="h0t")
        xw0t = const.tile([128, NM * BL * CHUNK], f32, tag="xw0t")
        xw1t = const.tile([128, NM * BL * CHUNK], f32, tag="xw1t")
        # recurrent state, col = k*BL + b
        c0 = const.tile([128, KT * BL], f32, tag="c0")
        c1 = const.tile([128, KT * BL], f32, tag="c1")
        h1 = const.tile([128, KT * BL], mdt, tag="h1")

        nc.sync.dma_start(xall[:, :].rearrange("p (b t) -> p b t", b=BL),
                          xprep_d[:, :, :])
        nc.sync.dma_start(w0[:, :], w0_d[:, :])
        for k in range(KT):
            sl = slice(k * 128, (k + 1) * 128)
            nc.sync.dma_start(u0[k][:, :], u0_d[sl, :])
            nc.sync.dma_start(w1[k][:, :], w1_d[sl, :])
            nc.sync.dma_start(u1[k][:, :], u1_d[sl, :])
            nc.sync.dma_start(wf[k][:, :], wfc_d[sl, :])
        nc.sync.dma_start(b0t[:, :], b0t_d[:, :])
        nc.sync.dma_start(b1t[:, :], b1t_d[:, :])
        nc.sync.dma_start(bfct[:, :], bfct_d[:, :])

        nc.vector.memset(h0t[:, :], 0.0)
        nc.vector.memset(c0[:, :], 0.0)
        nc.vector.memset(c1[:, :], 0.0)
        nc.vector.memset(h1[:, :], 0.0)

        def lstm_step(tl, uw, xwt, cst, h_rhs_fn, h_out_ap):
            """One recurrence step. h_rhs_fn(k)->[128,BL] prev-h AP,
            h_out_ap: [128, KT, BL] target for the new h."""
            zp = psum.tile([128, NM * BL], f32, tag="zp")
            for m in range(NM):
                msl = slice(m * 128, (m + 1) * 128)
                for k in range(KT):
                    nc.tensor.matmul(zp[:, m * BL:(m + 1) * BL],
                                     lhsT=uw[k][:, msl], rhs=h_rhs_fn(k),
                                     start=(k == 0), stop=(k == KT - 1))
            zs = work.tile([128, NM * BL], f32, tag="zs")
            xw_ap = xwt[:, :].rearrange("p (m b t) -> p m b t",
                                        m=NM, b=BL)[:, :, :, tl]
            nc.vector.tensor_tensor(
                zs[:, :].rearrange("p (m b) -> p m b", m=NM),
                zp[:, :].rearrange("p (m b) -> p m b", m=NM),
                xw_ap, ALU.add)
            # gate cols after host-side permutation: i 0:S, f S:2S,
            # o 2S:3S, g 3S:4S -- one sigmoid covers i,f,o
            S = KT * BL
            nc.scalar.activation(zs[:, 0:3 * S], zs[:, 0:3 * S], AF.Sigmoid)
            ig = work.tile([128, KT * BL], f32, tag="ig")
            nc.vector.tensor_tensor(ig[:, :], zs[:, 0:S],
                                    zs[:, 3 * S:4 * S], ALU.mult)
            nc.vector.tensor_tensor(cst[:, :], zs[:, S:2 * S],
                                    cst[:, :], ALU.mult)
            nc.vector.tensor_tensor(cst[:, :], cst[:, :], ig[:, :], ALU.add)
            nc.vector.tensor_tensor(
                h_out_ap,
                zs[:, 2 * S:3 * S].rearrange("p (k b) -> p k b", k=KT),
                cst[:, :].rearrange("p (k b) -> p k b", k=KT), ALU.mult)

        h0t_4d = h0t[:, :].rearrange("p (k b t) -> p k b t", k=KT, b=BL)

        with tc.For_i(0, T, CHUNK) as iv:
            # stage this chunk's x columns (only dynamic access in the body)
            xq = work.tile([128, BL * CHUNK], mdt, tag="xq")
            nc.vector.tensor_copy(
                xq[:, :].rearrange("p (b t) -> p b t", b=BL),
                xall[:, :].rearrange("p (b t) -> p b t",
                                     b=BL)[:, :, bass.ds(iv, CHUNK)])
            # xw0 = x @ W0 + b0 for the chunk
            for m in range(NM):
                msl = slice(m * 128, (m + 1) * 128)
                csl = slice(m * BL * CHUNK, (m + 1) * BL * CHUNK)
                psx = psum.tile([128, BL * CHUNK], f32, tag="psx")
                nc.tensor.matmul(psx[:, :], lhsT=w0[:, msl], rhs=xq[:, :],
                                 start=True, stop=True)
                nc.scalar.activation(xw0t[:, csl], psx[:, :], AF.Identity,
                                     bias=b0t[:, m:m + 1])
            # layer-0 recurrence; h stream written into h0t
            for tl in range(CHUNK):
                pv = (tl - 1) % CHUNK
                lstm_step(
                    tl, u0, xw0t, c0,
                    lambda k: h0t_4d[:, k, :, pv],
                    h0t_4d[:, :, :, tl])
            # xw1 = h0 @ W1 + b1 for the chunk
            for m in range(NM):
                msl = slice(m * 128, (m + 1) * 128)
                csl = slice(m * BL * CHUNK, (m + 1) * BL * CHUNK)
                psx = psum.tile([128, BL * CHUNK], f32, tag="psx")
                for k in range(KT):
                    nc.tensor.matmul(
                        psx[:, :], lhsT=w1[k][:, msl],
                        rhs=h0t[:, k * BL * CHUNK:(k + 1) * BL * CHUNK],
                        start=(k == 0), stop=(k == KT - 1))
                nc.scalar.activation(xw1t[:, csl], psx[:, :], AF.Identity,
                                     bias=b1t[:, m:m + 1])
            # layer-1 recurrence; only final h kept
            for tl in range(CHUNK):
                lstm_step(
                    tl, u1, xw1t, c1,
                    lambda k: h1[:, k * BL:(k + 1) * BL],
                    h1[:, :].rearrange("p (k b) -> p k b", k=KT))

        # FC head: out^T = Wfc^T @ h1^T + bfc
        psf = psum.tile([128, BL], f32, tag="psf")
        for k in range(KT):
            nc.tensor.matmul(psf[:, :], lhsT=wf[k][:, :],
                             rhs=h1[:, k * BL:(k + 1) * BL],
                             start=(k == 0), stop=(k == KT - 1))
        oT = work.tile([128, BL], f32, tag="oT")
        nc.scalar.activation(oT[:, :], psf[:, :], AF.Identity,
                             bias=bfct[:, 0:1])
        nc.sync.dma_start(out_d[:, :], oT[:, :])

    nc.compile()
    return nc


def _get_compiled():
    if "main" not in _cache:
        _cache["main"] = _build()
    return _cache["main"]


def _in_maps(input_seq, W0, U0, b0, W1, U1, b1, Wfc, bfc):
    mdt = _np_mmdt()
    x = np.asarray(input_seq, dtype=np.float32)
    # reorder gate blocks (i,f,g,o) -> (i,f,o,g) so one sigmoid instr
    # covers the first three
    perm = np.concatenate([np.arange(0, 2 * H),
                           np.arange(3 * H, 4 * H),
                           np.arange(2 * H, 3 * H)])

    def gp(w):
        return np.ascontiguousarray(
            np.asarray(w, np.float32)[..., perm].astype(mdt))

    shared = {
        "w0": gp(W0),
        "u0": gp(U0),
        "w1": gp(W1),
        "u1": gp(U1),
        "wfc": np.ascontiguousarray(np.asarray(Wfc, np.float32).astype(mdt)),
        "b0t": np.ascontiguousarray(
            np.asarray(b0, np.float32)[perm].reshape(NM, 128).T),
        "b1t": np.ascontiguousarray(
            np.asarray(b1, np.float32)[perm].reshape(NM, 128).T),
        "bfct": np.ascontiguousarray(np.asarray(bfc, np.float32).reshape(1, 128).T),
    }
    in_maps = []
    for c in range(NCORES):
        xs = x[c * BL:(c + 1) * BL]                       # [BL, T, I]
        xp = np.ascontiguousarray(xs.transpose(2, 0, 1).astype(mdt))
        m = dict(shared)
        m["xprep"] = xp
        in_maps.append(m)
    return in_maps


def _assemble(res):
    out = np.empty((B, 1, O), np.float32)
    for c in range(NCORES):
        out[c * BL:(c + 1) * BL, 0, :] = res.results[c]["outT"].T
    return out


def _run(nc, inputs):
    from concourse.bass_utils import run_bass_kernel_spmd
    in_maps = _in_maps(**inputs)
    res = run_bass_kernel_spmd(nc, in_maps, list(range(NCORES)))
    return _assemble(res)


def kernel(input_seq, W0, U0, b0, W1, U1, b1, Wfc, bfc):
    nc = _get_compiled()
    return _run(nc, dict(input_seq=input_seq, W0=W0, U0=U0, b0=b0, W1=W1,
                         U1=U1, b1=b1, Wfc=Wfc, bfc=bfc))



# revision 15
# speedup vs baseline: 1.5557x; 1.5557x over previous
"""Two-layer LSTM (linear cell/output activations) + FC head on 8 NeuronCores.

Strategy (data-parallel over batch, per the sharding hint):
  - B=32 split across 8 cores -> B_local=4 per core; weights replicated.
  - State transposed: h^T/c^T are [H on partitions, (k,b) on free]; the
    per-step recurrence is z^T += U^T @ h^T with U stationary, landing as
    [128, 4] column blocks in PSUM.
  - z lives ENTIRELY in PSUM: the per-chunk input projections (x@W0, h0@W1)
    and rank-1 bias matmuls write xw+b into PSUM up front; the per-step
    recurrence matmuls accumulate U@h on top (start=False). No DVE add.
  - PSUM layout: 4 banks per layer, bank = t%4 (parity). The engine reading
    z for step t (ACT) touches bank t%4 while the PE writes bank (t+1)%4 —
    never the same bank, so Tile's bank tracker keeps them parallel.
  - The two layers are software-pipelined ONE CHUNK apart: the loop body
    interleaves L0 step s of chunk i with L1 step s of chunk i-1. Each
    layer's sigmoid+gate-math chain hides under the other layer's matmul
    block. Prologue runs L0 chunk 0; epilogue runs L1 chunk 31 + FC.
  - Per-step chain: ACT copies g to SBUF + sigmoids i,f (early, after their
    m-blocks) and o; DVE does 4 small SBUF-only tensor_tensor ops. PE order
    per step: g-blocks, i, f, o so the sigmoid can start at ~75% of the
    matmul block.
"""

import os
import numpy as np
from contextlib import ExitStack

os.environ.setdefault("MYCRO_LOCAL_CACHE", "1")

B, T, I, H, O = 32, 2048, 128, 256, 128
NCORES = 8
BL = B // NCORES          # 4 batch elements per core
CHUNK = 64                # timesteps per loop body
G4 = 4 * H                # 1024 gate columns
NM = G4 // 128            # 8 gate m-blocks of 128
KT = H // 128             # 2 contraction tiles
NP = 4                    # PSUM bank parity groups
TG = CHUNK // NP          # 16 step-groups per parity
BKCOLS = TG * NM * BL     # 512 fp32 cols per bank

MM_BF16 = True

_cache = {}


def _np_mmdt():
    if MM_BF16:
        import ml_dtypes
        return ml_dtypes.bfloat16
    return np.float32


def _build(tiny=False):
    import concourse.bacc as bacc
    import concourse.bass as bass
    import concourse.tile as tile
    import concourse.mybir as mybir

    f32 = mybir.dt.float32
    mdt = mybir.dt.bfloat16 if MM_BF16 else f32
    AF = mybir.ActivationFunctionType
    ALU = mybir.AluOpType

    nc = bacc.Bacc("TRN2", target_bir_lowering=False, debug=False,
                   num_devices=NCORES)

    xprep_d = nc.declare_dram_parameter("xprep", [I, BL, T], mdt, isOutput=False)
    w0_d = nc.declare_dram_parameter("w0", [I, G4], mdt, isOutput=False)
    u0_d = nc.declare_dram_parameter("u0", [H, G4], mdt, isOutput=False)
    w1_d = nc.declare_dram_parameter("w1", [H, G4], mdt, isOutput=False)
    u1_d = nc.declare_dram_parameter("u1", [H, G4], mdt, isOutput=False)
    wfc_d = nc.declare_dram_parameter("wfc", [H, O], mdt, isOutput=False)
    b0r_d = nc.declare_dram_parameter("b0r", [128, 128], mdt, isOutput=False)
    b1r_d = nc.declare_dram_parameter("b1r", [128, 128], mdt, isOutput=False)
    oneh_d = nc.declare_dram_parameter("oneh", [128, BKCOLS], mdt,
                                       isOutput=False)
    bfct_d = nc.declare_dram_parameter("bfct", [128, 1], f32, isOutput=False)
    out_d = nc.declare_dram_parameter("outT", [O, BL], f32, isOutput=True)

    with tile.TileContext(nc) as tc, ExitStack() as ctx:
        if tiny:
            pool = ctx.enter_context(tc.tile_pool(name="tp", bufs=1))
            t1 = pool.tile([128, BL], mdt, tag="t1")
            t2 = pool.tile([128, BL], f32, tag="t2")
            nc.sync.dma_start(t1[:, :], xprep_d[:, :, 0])
            nc.vector.tensor_copy(t2[:, :], t1[:, :])
            nc.sync.dma_start(out_d[:, :], t2[:, :])
            nc.compile()
            return nc

        const = ctx.enter_context(tc.tile_pool(name="const", bufs=1))
        work = ctx.enter_context(tc.tile_pool(name="work", bufs=4))
        psum = ctx.enter_context(tc.tile_pool(name="psum", bufs=1, space="PSUM"))

        # ---- persistent SBUF residents ----
        # xall col = b*T + t
        xall = const.tile([128, BL * T], mdt, tag="xall")
        w0 = const.tile([128, G4], mdt, tag="w0")
        u0 = [const.tile([128, G4], mdt, tag=f"u0_{k}", name=f"u0_{k}")
              for k in range(KT)]
        w1 = [const.tile([128, G4], mdt, tag=f"w1_{k}", name=f"w1_{k}")
              for k in range(KT)]
        u1 = [const.tile([128, G4], mdt, tag=f"u1_{k}", name=f"u1_{k}")
              for k in range(KT)]
        wf = [const.tile([128, O], mdt, tag=f"wf_{k}", name=f"wf_{k}")
              for k in range(KT)]
        # bias matrices for the rank-1 bias matmuls: row r (partition r) =
        # bias of m-block r; block-one-hot rhs maps row m to col block m,
        # so ONE matmul adds every m-block's bias to a whole PSUM bank.
        ball0 = const.tile([128, 128], mdt, tag="ball0")
        ball1 = const.tile([128, 128], mdt, tag="ball1")
        onehot = const.tile([128, BKCOLS], mdt, tag="onehot")
        bfct = const.tile([128, 1], f32, tag="bfct")
        # h0 stream for one chunk, parity-major: col = k*256 + (s%4)*64
        #   + (s//4)*4 + b
        h0t = const.tile([128, KT * CHUNK * BL], mdt, tag="h0t")
        # recurrent state, col = k*BL + b
        c0 = const.tile([128, KT * BL], f32, tag="c0")
        c1 = const.tile([128, KT * BL], f32, tag="c1")
        h1 = const.tile([128, KT * BL], mdt, tag="h1")

        # ---- PSUM: z for each layer, 4 banks, bank = step parity ----
        # col = (s%4)*BKCOLS + m*TG*BL + (s//4)*BL + b
        zl0 = psum.tile([128, NP * BKCOLS], f32, tag="zl0")
        zl1 = psum.tile([128, NP * BKCOLS], f32, tag="zl1")

        nc.sync.dma_start(xall[:, :].rearrange("p (b t) -> p b t", b=BL),
                          xprep_d[:, :, :])
        nc.sync.dma_start(w0[:, :], w0_d[:, :])
        for k in range(KT):
            sl = slice(k * 128, (k + 1) * 128)
            nc.sync.dma_start(u0[k][:, :], u0_d[sl, :])
            nc.sync.dma_start(w1[k][:, :], w1_d[sl, :])
            nc.sync.dma_start(u1[k][:, :], u1_d[sl, :])
            nc.sync.dma_start(wf[k][:, :], wfc_d[sl, :])
        nc.sync.dma_start(ball0[:, :], b0r_d[:, :])
        nc.sync.dma_start(ball1[:, :], b1r_d[:, :])
        nc.sync.dma_start(bfct[:, :], bfct_d[:, :])
        nc.sync.dma_start(onehot[:, :], oneh_d[:, :])
        nc.vector.memset(h0t[:, :], 0.0)
        nc.vector.memset(c0[:, :], 0.0)
        nc.vector.memset(c1[:, :], 0.0)
        nc.vector.memset(h1[:, :], 0.0)

        # z view [p, parity, m, tg, b]; h0 view [p, k, parity, tg, b]
        zv0 = zl0[:, :].rearrange("p (pr m tg b) -> p pr m tg b",
                                  pr=NP, m=NM, tg=TG)
        zv1 = zl1[:, :].rearrange("p (pr m tg b) -> p pr m tg b",
                                  pr=NP, m=NM, tg=TG)
        h0v = h0t[:, :].rearrange("p (k pr tg b) -> p k pr tg b",
                                  k=KT, pr=NP, tg=TG)

        def zcols(zv, s, m):
            """z AP for step s, m-block m: [128, BL]."""
            return zv[:, s % NP, m, s // NP]

        def zgates(zv, s, m0, m1):
            """z AP for step s over m-blocks m0..m1: [128, nm, BL]."""
            return zv[:, s % NP, m0:m1, s // NP]

        def h0cols(s, k):
            return h0v[:, k, s % NP, s // NP]

        def h0ap_write(s):
            """[128, KT, BL] write AP for step s's h in h0t."""
            return h0v[:, :, s % NP, s // NP]

        # m-block order: g (6,7) first, then i (0,1), f (2,3), o (4,5)
        M_ORDER = [6, 7, 0, 1, 2, 3, 4, 5]

        def lstm_step(s, uw, zv, cst, h_rhs_fn, h_out_ap):
            for m in M_ORDER:
                for k in range(KT):
                    nc.tensor.matmul(zcols(zv, s, m),
                                     lhsT=uw[k][:, m * 128:(m + 1) * 128],
                                     rhs=h_rhs_fn(k),
                                     start=False, stop=(k == KT - 1))
            # ACT: evacuate g to SBUF, sigmoid i,f then o
            zg = work.tile([128, KT * BL], f32, tag="zg")
            nc.scalar.activation(zg[:, :].rearrange("p (k b) -> p k b", k=KT),
                                 zgates(zv, s, 6, 8), AF.Identity)
            sif = work.tile([128, 2 * KT * BL], f32, tag="sif")
            nc.scalar.activation(
                sif[:, :].rearrange("p (m b) -> p m b", m=2 * KT),
                zgates(zv, s, 0, 4), AF.Sigmoid)
            so = work.tile([128, KT * BL], f32, tag="so")
            nc.scalar.activation(so[:, :].rearrange("p (k b) -> p k b", k=KT),
                                 zgates(zv, s, 4, 6), AF.Sigmoid)
            # DVE: c = f*c + i*g ; h = o*c   (all SBUF operands)
            S = KT * BL
            ig = work.tile([128, S], f32, tag="ig")
            nc.vector.tensor_tensor(cst[:, :], sif[:, S:2 * S], cst[:, :],
                                    ALU.mult)
            nc.vector.tensor_tensor(ig[:, :], sif[:, 0:S], zg[:, :], ALU.mult)
            nc.vector.tensor_tensor(cst[:, :], cst[:, :], ig[:, :], ALU.add)
            nc.vector.tensor_tensor(
                h_out_ap,
                so[:, :].rearrange("p (k b) -> p k b", k=KT),
                cst[:, :].rearrange("p (k b) -> p k b", k=KT), ALU.mult)

        def proj_bias(zl, ball):
            """z += bias: one N=512 matmul per bank adds every m-block's
            bias (ball row m) to its col block via the block-one-hot rhs."""
            for p in range(NP):
                nc.tensor.matmul(
                    zl[:, p * BKCOLS:(p + 1) * BKCOLS],
                    lhsT=ball[:, :], rhs=onehot[:, :],
                    start=False, stop=False)

        def proj_l0(iv):
            """xw0 for chunk at t0=iv into zl0 (start=True clears banks)."""
            xq = work.tile([128, BL * CHUNK], mdt, tag="xq")
            # xq col = (t%4)*64 + (t//4)*4 + b  <- xall[b*T + iv + p + 4*tg]
            src = xall[:, :].rearrange("p (b t) -> p b t", b=BL)
            nc.vector.tensor_copy(
                xq[:, :].rearrange("p (pr tg b) -> p pr tg b", pr=NP, tg=TG),
                src[:, :, bass.ds(iv, CHUNK)].rearrange(
                    "p b (tg pr) -> p pr tg b", pr=NP),
            )
            for m in range(NM):
                for p in range(NP):
                    nc.tensor.matmul(
                        zl0[:, p * BKCOLS + m * TG * BL:
                            p * BKCOLS + (m + 1) * TG * BL],
                        lhsT=w0[:, m * 128:(m + 1) * 128],
                        rhs=xq[:, p * TG * BL:(p + 1) * TG * BL],
                        start=(m == 0), stop=False)
            proj_bias(zl0, ball0)

        def proj_l1():
            """xw1 = W1 @ h0(prev chunk) into zl1."""
            for m in range(NM):
                for p in range(NP):
                    for k in range(KT):
                        nc.tensor.matmul(
                            zl1[:, p * BKCOLS + m * TG * BL:
                                p * BKCOLS + (m + 1) * TG * BL],
                            lhsT=w1[k][:, m * 128:(m + 1) * 128],
                            rhs=h0t[:, k * CHUNK * BL + p * TG * BL:
                                    k * CHUNK * BL + (p + 1) * TG * BL],
                            start=(m == 0 and k == 0), stop=False)
            proj_bias(zl1, ball1)

        def l0_step(s):
            lstm_step(s, u0, zv0, c0,
                      lambda k, _s=s: h0cols((_s - 1) % CHUNK, k),
                      h0ap_write(s))

        def l1_step(s):
            lstm_step(s, u1, zv1, c1,
                      lambda k: h1[:, k * BL:(k + 1) * BL],
                      h1[:, :].rearrange("p (k b) -> p k b", k=KT))

        # ---- prologue: L0 chunk 0 ----
        proj_l0(0)
        for s in range(CHUNK):
            l0_step(s)

        # ---- main loop: L0 chunk i (t0=iv), L1 chunk i-1 ----
        with tc.For_i(CHUNK, T, CHUNK) as iv:
            proj_l1()
            proj_l0(iv)
            for s in range(CHUNK):
                l0_step(s)
                l1_step(s)

        # ---- epilogue: L1 chunk 31, FC head ----
        proj_l1()
        for s in range(CHUNK):
            l1_step(s)

        psf = zl0[:, 0:BL]
        for k in range(KT):
            nc.tensor.matmul(psf, lhsT=wf[k][:, :],
                             rhs=h1[:, k * BL:(k + 1) * BL],
                             start=(k == 0), stop=(k == KT - 1))
        oT = work.tile([128, BL], f32, tag="oT")
        nc.scalar.activation(oT[:, :], psf, AF.Identity, bias=bfct[:, 0:1])
        nc.sync.dma_start(out_d[:, :], oT[:, :])

    nc.compile()
    return nc


def _get_compiled():
    if "main" not in _cache:
        _cache["main"] = _build()
    return _cache["main"]


def _ballmat(b, perm, mdt):
    m = np.zeros((128, 128), np.float32)
    m[0:NM, :] = np.asarray(b, np.float32)[perm].reshape(NM, 128)
    return np.ascontiguousarray(m.astype(mdt))


def _onehot(mdt):
    m = np.zeros((128, BKCOLS), np.float32)
    for r in range(NM):
        m[r, r * TG * BL:(r + 1) * TG * BL] = 1.0
    return np.ascontiguousarray(m.astype(mdt))


def _in_maps(input_seq, W0, U0, b0, W1, U1, b1, Wfc, bfc):
    mdt = _np_mmdt()
    x = np.asarray(input_seq, dtype=np.float32)
    # reorder gate blocks (i,f,g,o) -> (i,f,o,g)
    perm = np.concatenate([np.arange(0, 2 * H),
                           np.arange(3 * H, 4 * H),
                           np.arange(2 * H, 3 * H)])

    def gp(w):
        return np.ascontiguousarray(
            np.asarray(w, np.float32)[..., perm].astype(mdt))

    shared = {
        "w0": gp(W0),
        "u0": gp(U0),
        "w1": gp(W1),
        "u1": gp(U1),
        "wfc": np.ascontiguousarray(np.asarray(Wfc, np.float32).astype(mdt)),
        "b0r": _ballmat(b0, perm, mdt),
        "b1r": _ballmat(b1, perm, mdt),
        "oneh": _onehot(mdt),
        "bfct": np.ascontiguousarray(
            np.asarray(bfc, np.float32).reshape(1, 128).T),
    }
    in_maps = []
    for c in range(NCORES):
        xs = x[c * BL:(c + 1) * BL]                       # [BL, T, I]
        xp = np.ascontiguousarray(xs.transpose(2, 0, 1).astype(mdt))
        m = dict(shared)
        m["xprep"] = xp
        in_maps.append(m)
    return in_maps


def _assemble(res):
    out = np.empty((B, 1, O), np.float32)
    for c in range(NCORES):
        out[c * BL:(c + 1) * BL, 0, :] = res.results[c]["outT"].T
    return out


def _run(nc, inputs):
    from concourse.bass_utils import run_bass_kernel_spmd
    in_maps = _in_maps(**inputs)
    res = run_bass_kernel_spmd(nc, in_maps, list(range(NCORES)))
    return _assemble(res)


def kernel(input_seq, W0, U0, b0, W1, U1, b1, Wfc, bfc):
    nc = _get_compiled()
    return _run(nc, dict(input_seq=input_seq, W0=W0, U0=U0, b0=b0, W1=W1,
                         U1=U1, b1=b1, Wfc=Wfc, bfc=bfc))


# revision 21
# speedup vs baseline: 1.7902x; 1.1507x over previous
"""Two-layer LSTM (linear cell/output activations) + FC head on 8 NeuronCores.

Strategy (data-parallel over batch, per the sharding hint):
  - B=32 split across 8 cores -> B_local=4 per core; weights replicated.
  - State transposed: h^T/c^T are [H on partitions, (k,b) on free]; the
    per-step recurrence is z^T += U^T @ h^T with U stationary, landing as
    [128, 4] column blocks in PSUM.
  - z lives ENTIRELY in PSUM: the per-chunk input projections (x@W0, h0@W1)
    and rank-1 bias matmuls write xw+b into PSUM up front; the per-step
    recurrence matmuls accumulate U@h on top (start=False). No DVE add.
  - PSUM layout: 4 banks per layer, bank = t%4 (parity). The engine reading
    z for step t (ACT) touches bank t%4 while the PE writes bank (t+1)%4 —
    never the same bank, so Tile's bank tracker keeps them parallel.
  - The two layers are software-pipelined ONE CHUNK apart: the loop body
    interleaves L0 step s of chunk i with L1 step s of chunk i-1. Each
    layer's sigmoid+gate-math chain hides under the other layer's matmul
    block. Prologue runs L0 chunk 0; epilogue runs L1 chunk 31 + FC.
  - Per-step chain: ACT copies g to SBUF + sigmoids i,f (early, after their
    m-blocks) and o; DVE does 4 small SBUF-only tensor_tensor ops. PE order
    per step: g-blocks, i, f, o so the sigmoid can start at ~75% of the
    matmul block.
"""

import os
import numpy as np
from contextlib import ExitStack

os.environ.setdefault("MYCRO_LOCAL_CACHE", "1")

B, T, I, H, O = 32, 2048, 128, 256, 128
NCORES = 8
BL = B // NCORES          # 4 batch elements per core
CHUNK = 64                # timesteps per loop body
G4 = 4 * H                # 1024 gate columns
NM = G4 // 128            # 8 gate m-blocks of 128
KT = H // 128             # 2 contraction tiles
NP = 4                    # PSUM bank parity groups
TG = CHUNK // NP          # 16 step-groups per parity
BKCOLS = TG * NM * BL     # 512 fp32 cols per bank

MM_BF16 = True

_cache = {}


def _np_mmdt():
    if MM_BF16:
        import ml_dtypes
        return ml_dtypes.bfloat16
    return np.float32


def _build(tiny=False):
    import concourse.bacc as bacc
    import concourse.bass as bass
    import concourse.tile as tile
    import concourse.mybir as mybir

    f32 = mybir.dt.float32
    mdt = mybir.dt.bfloat16 if MM_BF16 else f32
    AF = mybir.ActivationFunctionType
    ALU = mybir.AluOpType

    nc = bacc.Bacc("TRN2", target_bir_lowering=False, debug=False,
                   num_devices=NCORES)

    xprep_d = nc.declare_dram_parameter("xprep", [I, BL, T], mdt, isOutput=False)
    w0_d = nc.declare_dram_parameter("w0", [I, G4], mdt, isOutput=False)
    u0_d = nc.declare_dram_parameter("u0", [H, G4], mdt, isOutput=False)
    w1_d = nc.declare_dram_parameter("w1", [H, G4], mdt, isOutput=False)
    u1_d = nc.declare_dram_parameter("u1", [H, G4], mdt, isOutput=False)
    wfc_d = nc.declare_dram_parameter("wfc", [H, O], mdt, isOutput=False)
    b0r_d = nc.declare_dram_parameter("b0r", [128, 128], mdt, isOutput=False)
    b1r_d = nc.declare_dram_parameter("b1r", [128, 128], mdt, isOutput=False)
    oneh_d = nc.declare_dram_parameter("oneh", [128, BKCOLS], mdt,
                                       isOutput=False)
    bfct_d = nc.declare_dram_parameter("bfct", [128, 1], f32, isOutput=False)
    out_d = nc.declare_dram_parameter("outT", [O, BL], f32, isOutput=True)

    with tile.TileContext(nc) as tc, ExitStack() as ctx:
        if tiny:
            pool = ctx.enter_context(tc.tile_pool(name="tp", bufs=1))
            t1 = pool.tile([128, BL], mdt, tag="t1")
            t2 = pool.tile([128, BL], f32, tag="t2")
            nc.sync.dma_start(t1[:, :], xprep_d[:, :, 0])
            nc.vector.tensor_copy(t2[:, :], t1[:, :])
            nc.sync.dma_start(out_d[:, :], t2[:, :])
            nc.compile()
            return nc

        const = ctx.enter_context(tc.tile_pool(name="const", bufs=1))
        work = ctx.enter_context(tc.tile_pool(name="work", bufs=4))
        psum = ctx.enter_context(tc.tile_pool(name="psum", bufs=1, space="PSUM"))

        # ---- persistent SBUF residents ----
        # xall col = b*T + t
        xall = const.tile([128, BL * T], mdt, tag="xall")
        w0 = const.tile([128, G4], mdt, tag="w0")
        u0 = [const.tile([128, G4], mdt, tag=f"u0_{k}", name=f"u0_{k}")
              for k in range(KT)]
        w1 = [const.tile([128, G4], mdt, tag=f"w1_{k}", name=f"w1_{k}")
              for k in range(KT)]
        u1 = [const.tile([128, G4], mdt, tag=f"u1_{k}", name=f"u1_{k}")
              for k in range(KT)]
        wf = [const.tile([128, O], mdt, tag=f"wf_{k}", name=f"wf_{k}")
              for k in range(KT)]
        # bias matrices for the rank-1 bias matmuls: row r (partition r) =
        # bias of m-block r; block-one-hot rhs maps row m to col block m,
        # so ONE matmul adds every m-block's bias to a whole PSUM bank.
        ball0 = const.tile([128, 128], mdt, tag="ball0")
        ball1 = const.tile([128, 128], mdt, tag="ball1")
        onehot = const.tile([128, BKCOLS], mdt, tag="onehot")
        bfct = const.tile([128, 1], f32, tag="bfct")
        # h0 stream for one chunk, parity-major: col = k*256 + (s%4)*64
        #   + (s//4)*4 + b
        h0t = const.tile([128, KT * CHUNK * BL], mdt, tag="h0t")
        # recurrent state, col = k*BL + b
        c0 = const.tile([128, KT * BL], mdt, tag="c0")
        c1 = const.tile([128, KT * BL], mdt, tag="c1")
        h1 = const.tile([128, KT * BL], mdt, tag="h1")

        # ---- PSUM: z for each layer, 4 banks, bank = step parity ----
        # col = (s%4)*BKCOLS + m*TG*BL + (s//4)*BL + b
        zl0 = psum.tile([128, NP * BKCOLS], f32, tag="zl0")
        zl1 = psum.tile([128, NP * BKCOLS], f32, tag="zl1")

        nc.sync.dma_start(xall[:, :].rearrange("p (b t) -> p b t", b=BL),
                          xprep_d[:, :, :])
        nc.sync.dma_start(w0[:, :], w0_d[:, :])
        for k in range(KT):
            sl = slice(k * 128, (k + 1) * 128)
            nc.sync.dma_start(u0[k][:, :], u0_d[sl, :])
            nc.sync.dma_start(w1[k][:, :], w1_d[sl, :])
            nc.sync.dma_start(u1[k][:, :], u1_d[sl, :])
            nc.sync.dma_start(wf[k][:, :], wfc_d[sl, :])
        nc.sync.dma_start(ball0[:, :], b0r_d[:, :])
        nc.sync.dma_start(ball1[:, :], b1r_d[:, :])
        nc.sync.dma_start(bfct[:, :], bfct_d[:, :])
        nc.sync.dma_start(onehot[:, :], oneh_d[:, :])
        nc.vector.memset(h0t[:, :], 0.0)
        nc.vector.memset(c0[:, :], 0.0)
        nc.vector.memset(c1[:, :], 0.0)
        nc.vector.memset(h1[:, :], 0.0)

        # z bank layout is tg-major: col = pr*BKCOLS + tg*(NM*BL) + m*BL + b
        # so one step's z is 32 CONTIGUOUS fp32 (i,f,o = 24, g = 8).
        # h0 layout: col = k*CHUNK*BL + pr*TG*BL + tg*BL + b
        h0v = h0t[:, :].rearrange("p (k pr tg b) -> p k pr tg b",
                                  k=KT, pr=NP, tg=TG)

        SW = NM * BL          # 32 z cols per step

        def zstep(zl, s):
            """offset of step s's z block (32 cols)."""
            return (s % NP) * BKCOLS + (s // NP) * SW

        def zcols(zl, s, m):
            """z AP for step s, m-block m: [128, BL]."""
            base = zstep(zl, s) + m * BL
            return zl[:, base:base + BL]

        def h0cols(s, k):
            return h0v[:, k, s % NP, s // NP]

        def h0ap_write(s):
            """[128, KT, BL] write AP for step s's h in h0t."""
            return h0v[:, :, s % NP, s // NP]

        # m-block order: g (6,7) first, then i (0,1), f (2,3), o (4,5)
        M_ORDER = [6, 7, 0, 1, 2, 3, 4, 5]

        def lstm_step(s, uw, zl, cst, h_rhs_fn, h_out_ap):
            S = KT * BL
            for m in M_ORDER:
                for k in range(KT):
                    nc.tensor.matmul(zcols(zl, s, m),
                                     lhsT=uw[k][:, m * 128:(m + 1) * 128],
                                     rhs=h_rhs_fn(k),
                                     start=False, stop=(k == KT - 1))
            zb = zstep(zl, s)
            # DVE evacuates g early (bf16); one sigmoid covers i,f,o
            zg = work.tile([128, S], mdt, tag="zg")
            nc.vector.tensor_copy(zg[:, :], zl[:, zb + 3 * S:zb + 4 * S])
            sg = work.tile([128, 3 * S], mdt, tag="sg")
            nc.scalar.activation(sg[:, :], zl[:, zb:zb + 3 * S], AF.Sigmoid)
            # DVE: c = f*c + i*g ; h = o*c  (bf16 SBUF -> 2x mode)
            ig = work.tile([128, S], mdt, tag="ig")
            nc.vector.tensor_tensor(cst[:, :], sg[:, S:2 * S], cst[:, :],
                                    ALU.mult)
            nc.vector.tensor_tensor(ig[:, :], sg[:, 0:S], zg[:, :], ALU.mult)
            nc.vector.tensor_tensor(cst[:, :], cst[:, :], ig[:, :], ALU.add)
            nc.vector.tensor_tensor(
                h_out_ap,
                sg[:, 2 * S:3 * S].rearrange("p (k b) -> p k b", k=KT),
                cst[:, :].rearrange("p (k b) -> p k b", k=KT), ALU.mult)

        def proj_bias(zl, ball):
            """z += bias: one N=512 matmul per bank adds every m-block's
            bias (ball row m) to its col block via the block-one-hot rhs."""
            for p in range(NP):
                nc.tensor.matmul(
                    zl[:, p * BKCOLS:(p + 1) * BKCOLS],
                    lhsT=ball[:, :], rhs=onehot[:, :],
                    start=False, stop=False)

        # proj-output view [p, pr, tg, m, b] (strided per m-block)
        zp0 = zl0[:, :].rearrange("p (pr tg m b) -> p pr tg m b",
                                  pr=NP, tg=TG, m=NM)
        zp1 = zl1[:, :].rearrange("p (pr tg m b) -> p pr tg m b",
                                  pr=NP, tg=TG, m=NM)

        def proj_l0(iv):
            """xw0 for chunk at t0=iv into zl0 (start=True clears banks)."""
            xq = work.tile([128, BL * CHUNK], mdt, tag="xq")
            # xq col = (t%4)*64 + (t//4)*4 + b  <- xall[b*T + iv + p + 4*tg]
            src = xall[:, :].rearrange("p (b t) -> p b t", b=BL)
            nc.vector.tensor_copy(
                xq[:, :].rearrange("p (pr tg b) -> p pr tg b", pr=NP, tg=TG),
                src[:, :, bass.ds(iv, CHUNK)].rearrange(
                    "p b (tg pr) -> p pr tg b", pr=NP),
            )
            for m in range(NM):
                for p in range(NP):
                    nc.tensor.matmul(
                        zp0[:, p, :, m],
                        lhsT=w0[:, m * 128:(m + 1) * 128],
                        rhs=xq[:, p * TG * BL:(p + 1) * TG * BL],
                        start=(m == 0), stop=False)
            proj_bias(zl0, ball0)

        def proj_l1():
            """xw1 = W1 @ h0(prev chunk) into zl1."""
            for m in range(NM):
                for p in range(NP):
                    for k in range(KT):
                        nc.tensor.matmul(
                            zp1[:, p, :, m],
                            lhsT=w1[k][:, m * 128:(m + 1) * 128],
                            rhs=h0t[:, k * CHUNK * BL + p * TG * BL:
                                    k * CHUNK * BL + (p + 1) * TG * BL],
                            start=(m == 0 and k == 0), stop=False)
            proj_bias(zl1, ball1)

        def l0_step(s):
            lstm_step(s, u0, zl0, c0,
                      lambda k, _s=s: h0cols((_s - 1) % CHUNK, k),
                      h0ap_write(s))

        def l1_step(s):
            lstm_step(s, u1, zl1, c1,
                      lambda k: h1[:, k * BL:(k + 1) * BL],
                      h1[:, :].rearrange("p (k b) -> p k b", k=KT))

        # ---- prologue: L0 chunk 0 ----
        proj_l0(0)
        for s in range(CHUNK):
            l0_step(s)

        # ---- main loop: L0 chunk i (t0=iv), L1 chunk i-1 ----
        with tc.For_i(CHUNK, T, CHUNK) as iv:
            proj_l1()
            proj_l0(iv)
            for s in range(CHUNK):
                l0_step(s)
                l1_step(s)

        # ---- epilogue: L1 chunk 31, FC head ----
        proj_l1()
        for s in range(CHUNK):
            l1_step(s)

        psf = zl0[:, 0:BL]
        for k in range(KT):
            nc.tensor.matmul(psf, lhsT=wf[k][:, :],
                             rhs=h1[:, k * BL:(k + 1) * BL],
                             start=(k == 0), stop=(k == KT - 1))
        oT = work.tile([128, BL], f32, tag="oT")
        nc.scalar.activation(oT[:, :], psf, AF.Identity, bias=bfct[:, 0:1])
        nc.sync.dma_start(out_d[:, :], oT[:, :])

    nc.compile()
    return nc


def _get_compiled():
    if "main" not in _cache:
        _cache["main"] = _build()
    return _cache["main"]


def _ballmat(b, perm, mdt):
    m = np.zeros((128, 128), np.float32)
    m[0:NM, :] = np.asarray(b, np.float32)[perm].reshape(NM, 128)
    return np.ascontiguousarray(m.astype(mdt))


def _onehot(mdt):
    # z bank layout is tg-major: col = tg*(NM*BL) + m*BL + b -> row m hot
    # wherever (col % (NM*BL)) // BL == m
    m = np.zeros((128, BKCOLS), np.float32)
    cols = np.arange(BKCOLS)
    m[(cols % (NM * BL)) // BL, cols] = 1.0
    return np.ascontiguousarray(m.astype(mdt))


def _in_maps(input_seq, W0, U0, b0, W1, U1, b1, Wfc, bfc):
    mdt = _np_mmdt()
    x = np.asarray(input_seq, dtype=np.float32)
    # reorder gate blocks (i,f,g,o) -> (i,f,o,g)
    perm = np.concatenate([np.arange(0, 2 * H),
                           np.arange(3 * H, 4 * H),
                           np.arange(2 * H, 3 * H)])

    def gp(w):
        return np.ascontiguousarray(
            np.asarray(w, np.float32)[..., perm].astype(mdt))

    shared = {
        "w0": gp(W0),
        "u0": gp(U0),
        "w1": gp(W1),
        "u1": gp(U1),
        "wfc": np.ascontiguousarray(np.asarray(Wfc, np.float32).astype(mdt)),
        "b0r": _ballmat(b0, perm, mdt),
        "b1r": _ballmat(b1, perm, mdt),
        "oneh": _onehot(mdt),
        "bfct": np.ascontiguousarray(
            np.asarray(bfc, np.float32).reshape(1, 128).T),
    }
    in_maps = []
    for c in range(NCORES):
        xs = x[c * BL:(c + 1) * BL]                       # [BL, T, I]
        xp = np.ascontiguousarray(xs.transpose(2, 0, 1).astype(mdt))
        m = dict(shared)
        m["xprep"] = xp
        in_maps.append(m)
    return in_maps


def _assemble(res):
    out = np.empty((B, 1, O), np.float32)
    for c in range(NCORES):
        out[c * BL:(c + 1) * BL, 0, :] = res.results[c]["outT"].T
    return out


def _run(nc, inputs):
    from concourse.bass_utils import run_bass_kernel_spmd
    in_maps = _in_maps(**inputs)
    res = run_bass_kernel_spmd(nc, in_maps, list(range(NCORES)))
    return _assemble(res)


def kernel(input_seq, W0, U0, b0, W1, U1, b1, Wfc, bfc):
    nc = _get_compiled()
    return _run(nc, dict(input_seq=input_seq, W0=W0, U0=U0, b0=b0, W1=W1,
                         U1=U1, b1=b1, Wfc=Wfc, bfc=bfc))


# revision 23
# speedup vs baseline: 1.8208x; 1.0171x over previous
"""Two-layer LSTM (linear cell/output activations) + FC head on 8 NeuronCores.

Strategy (data-parallel over batch, per the sharding hint):
  - B=32 split across 8 cores -> B_local=4 per core; weights replicated.
  - State transposed: h^T/c^T are [H on partitions, (k,b) on free]; the
    per-step recurrence is z^T += U^T @ h^T with U stationary, landing as
    [128, 4] column blocks in PSUM.
  - z lives ENTIRELY in PSUM: the per-chunk input projections (x@W0, h0@W1)
    and rank-1 bias matmuls write xw+b into PSUM up front; the per-step
    recurrence matmuls accumulate U@h on top (start=False). No DVE add.
  - PSUM layout: 4 banks per layer, bank = t%4 (parity). The engine reading
    z for step t (ACT) touches bank t%4 while the PE writes bank (t+1)%4 —
    never the same bank, so Tile's bank tracker keeps them parallel.
  - The two layers are software-pipelined ONE CHUNK apart: the loop body
    interleaves L0 step s of chunk i with L1 step s of chunk i-1. Each
    layer's sigmoid+gate-math chain hides under the other layer's matmul
    block. Prologue runs L0 chunk 0; epilogue runs L1 chunk 31 + FC.
  - Per-step chain: ACT copies g to SBUF + sigmoids i,f (early, after their
    m-blocks) and o; DVE does 4 small SBUF-only tensor_tensor ops. PE order
    per step: g-blocks, i, f, o so the sigmoid can start at ~75% of the
    matmul block.
"""

import os
import numpy as np
from contextlib import ExitStack

os.environ.setdefault("MYCRO_LOCAL_CACHE", "1")

B, T, I, H, O = 32, 2048, 128, 256, 128
NCORES = 8
BL = B // NCORES          # 4 batch elements per core
CHUNK = 64                # timesteps per loop body
G4 = 4 * H                # 1024 gate columns
NM = G4 // 128            # 8 gate m-blocks of 128
KT = H // 128             # 2 contraction tiles
NP = 4                    # PSUM bank parity groups
TG = CHUNK // NP          # 16 step-groups per parity
BKCOLS = TG * NM * BL     # 512 fp32 cols per bank

MM_BF16 = True

_cache = {}


def _np_mmdt():
    if MM_BF16:
        import ml_dtypes
        return ml_dtypes.bfloat16
    return np.float32


def _build(tiny=False):
    import concourse.bacc as bacc
    import concourse.bass as bass
    import concourse.tile as tile
    import concourse.mybir as mybir

    f32 = mybir.dt.float32
    mdt = mybir.dt.bfloat16 if MM_BF16 else f32
    AF = mybir.ActivationFunctionType
    ALU = mybir.AluOpType

    nc = bacc.Bacc("TRN2", target_bir_lowering=False, debug=False,
                   num_devices=NCORES)

    xprep_d = nc.declare_dram_parameter("xprep", [I, BL, T], mdt, isOutput=False)
    w0_d = nc.declare_dram_parameter("w0", [I, G4], mdt, isOutput=False)
    u0_d = nc.declare_dram_parameter("u0", [H, G4], mdt, isOutput=False)
    w1_d = nc.declare_dram_parameter("w1", [H, G4], mdt, isOutput=False)
    u1_d = nc.declare_dram_parameter("u1", [H, G4], mdt, isOutput=False)
    wfc_d = nc.declare_dram_parameter("wfc", [H, O], mdt, isOutput=False)
    b0r_d = nc.declare_dram_parameter("b0r", [128, 128], mdt, isOutput=False)
    b1r_d = nc.declare_dram_parameter("b1r", [128, 128], mdt, isOutput=False)
    oneh_d = nc.declare_dram_parameter("oneh", [128, BKCOLS], mdt,
                                       isOutput=False)
    bfct_d = nc.declare_dram_parameter("bfct", [128, 1], f32, isOutput=False)
    out_d = nc.declare_dram_parameter("outT", [O, BL], f32, isOutput=True)

    with tile.TileContext(nc) as tc, ExitStack() as ctx:
        if tiny:
            pool = ctx.enter_context(tc.tile_pool(name="tp", bufs=1))
            t1 = pool.tile([128, BL], mdt, tag="t1")
            t2 = pool.tile([128, BL], f32, tag="t2")
            nc.sync.dma_start(t1[:, :], xprep_d[:, :, 0])
            nc.vector.tensor_copy(t2[:, :], t1[:, :])
            nc.sync.dma_start(out_d[:, :], t2[:, :])
            nc.compile()
            return nc

        const = ctx.enter_context(tc.tile_pool(name="const", bufs=1))
        work = ctx.enter_context(tc.tile_pool(name="work", bufs=4))
        psum = ctx.enter_context(tc.tile_pool(name="psum", bufs=1, space="PSUM"))

        # ---- persistent SBUF residents ----
        # xall col = b*T + t
        xall = const.tile([128, BL * T], mdt, tag="xall")
        w0 = const.tile([128, G4], mdt, tag="w0")
        u0 = [const.tile([128, G4], mdt, tag=f"u0_{k}", name=f"u0_{k}")
              for k in range(KT)]
        w1 = [const.tile([128, G4], mdt, tag=f"w1_{k}", name=f"w1_{k}")
              for k in range(KT)]
        u1 = [const.tile([128, G4], mdt, tag=f"u1_{k}", name=f"u1_{k}")
              for k in range(KT)]
        wf = [const.tile([128, O], mdt, tag=f"wf_{k}", name=f"wf_{k}")
              for k in range(KT)]
        # bias matrices for the rank-1 bias matmuls: row r (partition r) =
        # bias of m-block r; block-one-hot rhs maps row m to col block m,
        # so ONE matmul adds every m-block's bias to a whole PSUM bank.
        ball0 = const.tile([128, 128], mdt, tag="ball0")
        ball1 = const.tile([128, 128], mdt, tag="ball1")
        onehot = const.tile([128, BKCOLS], mdt, tag="onehot")
        bfct = const.tile([128, 1], f32, tag="bfct")
        # h0 stream for one chunk, parity-major: col = k*256 + (s%4)*64
        #   + (s//4)*4 + b
        h0t = const.tile([128, KT * CHUNK * BL], mdt, tag="h0t")
        # recurrent state, col = k*BL + b
        c0 = const.tile([128, KT * BL], f32, tag="c0")
        c1 = const.tile([128, KT * BL], f32, tag="c1")
        h1 = const.tile([128, KT * BL], mdt, tag="h1")

        # ---- PSUM: z for each layer, 4 banks, bank = step parity ----
        # col = (s%4)*BKCOLS + m*TG*BL + (s//4)*BL + b
        zl0 = psum.tile([128, NP * BKCOLS], f32, tag="zl0")
        zl1 = psum.tile([128, NP * BKCOLS], f32, tag="zl1")

        nc.sync.dma_start(xall[:, :].rearrange("p (b t) -> p b t", b=BL),
                          xprep_d[:, :, :])
        nc.sync.dma_start(w0[:, :], w0_d[:, :])
        for k in range(KT):
            sl = slice(k * 128, (k + 1) * 128)
            nc.sync.dma_start(u0[k][:, :], u0_d[sl, :])
            nc.sync.dma_start(w1[k][:, :], w1_d[sl, :])
            nc.sync.dma_start(u1[k][:, :], u1_d[sl, :])
            nc.sync.dma_start(wf[k][:, :], wfc_d[sl, :])
        nc.sync.dma_start(ball0[:, :], b0r_d[:, :])
        nc.sync.dma_start(ball1[:, :], b1r_d[:, :])
        nc.sync.dma_start(bfct[:, :], bfct_d[:, :])
        nc.sync.dma_start(onehot[:, :], oneh_d[:, :])
        nc.vector.memset(h0t[:, :], 0.0)
        nc.vector.memset(c0[:, :], 0.0)
        nc.vector.memset(c1[:, :], 0.0)
        nc.vector.memset(h1[:, :], 0.0)

        # z bank layout is tg-major: col = pr*BKCOLS + tg*(NM*BL) + m*BL + b
        # so one step's z is 32 CONTIGUOUS fp32 (i,f,o = 24, g = 8).
        # h0 layout: col = k*CHUNK*BL + pr*TG*BL + tg*BL + b
        h0v = h0t[:, :].rearrange("p (k pr tg b) -> p k pr tg b",
                                  k=KT, pr=NP, tg=TG)

        SW = NM * BL          # 32 z cols per step

        def zstep(zl, s):
            """offset of step s's z block (32 cols)."""
            return (s % NP) * BKCOLS + (s // NP) * SW

        def zcols(zl, s, m):
            """z AP for step s, m-block m: [128, BL]."""
            base = zstep(zl, s) + m * BL
            return zl[:, base:base + BL]

        def h0cols(s, k):
            return h0v[:, k, s % NP, s // NP]

        def h0ap_write(s):
            """[128, KT, BL] write AP for step s's h in h0t."""
            return h0v[:, :, s % NP, s // NP]

        # m-block order: g (6,7) first, then i (0,1), f (2,3), o (4,5)
        M_ORDER = [6, 7, 0, 1, 2, 3, 4, 5]

        def lstm_step(s, uw, zl, cst, h_rhs_fn, h_out_ap):
            S = KT * BL
            for m in M_ORDER:
                for k in range(KT):
                    nc.tensor.matmul(zcols(zl, s, m),
                                     lhsT=uw[k][:, m * 128:(m + 1) * 128],
                                     rhs=h_rhs_fn(k),
                                     start=False, stop=(k == KT - 1))
            zb = zstep(zl, s)
            # one sigmoid covers i,f,o; g is consumed straight from PSUM
            sg = work.tile([128, 3 * S], f32, tag="sg")
            nc.scalar.activation(sg[:, :], zl[:, zb:zb + 3 * S], AF.Sigmoid)
            # DVE: c = f*c + i*g ; h = o*c
            ig = work.tile([128, S], f32, tag="ig")
            nc.vector.tensor_tensor(cst[:, :], sg[:, S:2 * S], cst[:, :],
                                    ALU.mult)
            nc.vector.tensor_tensor(ig[:, :], sg[:, 0:S],
                                    zl[:, zb + 3 * S:zb + 4 * S], ALU.mult)
            nc.vector.tensor_tensor(cst[:, :], cst[:, :], ig[:, :], ALU.add)
            nc.vector.tensor_tensor(
                h_out_ap,
                sg[:, 2 * S:3 * S].rearrange("p (k b) -> p k b", k=KT),
                cst[:, :].rearrange("p (k b) -> p k b", k=KT), ALU.mult)

        def proj_bias(zl, ball):
            """z += bias: one N=512 matmul per bank adds every m-block's
            bias (ball row m) to its col block via the block-one-hot rhs."""
            for p in range(NP):
                nc.tensor.matmul(
                    zl[:, p * BKCOLS:(p + 1) * BKCOLS],
                    lhsT=ball[:, :], rhs=onehot[:, :],
                    start=False, stop=False)

        # proj-output view [p, pr, tg, m, b] (strided per m-block)
        zp0 = zl0[:, :].rearrange("p (pr tg m b) -> p pr tg m b",
                                  pr=NP, tg=TG, m=NM)
        zp1 = zl1[:, :].rearrange("p (pr tg m b) -> p pr tg m b",
                                  pr=NP, tg=TG, m=NM)

        def proj_l0(iv):
            """xw0 for chunk at t0=iv into zl0 (start=True clears banks)."""
            xq = work.tile([128, BL * CHUNK], mdt, tag="xq")
            # xq col = (t%4)*64 + (t//4)*4 + b  <- xall[b*T + iv + p + 4*tg]
            src = xall[:, :].rearrange("p (b t) -> p b t", b=BL)
            nc.vector.tensor_copy(
                xq[:, :].rearrange("p (pr tg b) -> p pr tg b", pr=NP, tg=TG),
                src[:, :, bass.ds(iv, CHUNK)].rearrange(
                    "p b (tg pr) -> p pr tg b", pr=NP),
            )
            for m in range(NM):
                for p in range(NP):
                    nc.tensor.matmul(
                        zp0[:, p, :, m],
                        lhsT=w0[:, m * 128:(m + 1) * 128],
                        rhs=xq[:, p * TG * BL:(p + 1) * TG * BL],
                        start=(m == 0), stop=False)
            proj_bias(zl0, ball0)

        def proj_l1():
            """xw1 = W1 @ h0(prev chunk) into zl1."""
            for m in range(NM):
                for p in range(NP):
                    for k in range(KT):
                        nc.tensor.matmul(
                            zp1[:, p, :, m],
                            lhsT=w1[k][:, m * 128:(m + 1) * 128],
                            rhs=h0t[:, k * CHUNK * BL + p * TG * BL:
                                    k * CHUNK * BL + (p + 1) * TG * BL],
                            start=(m == 0 and k == 0), stop=False)
            proj_bias(zl1, ball1)

        def l0_step(s):
            lstm_step(s, u0, zl0, c0,
                      lambda k, _s=s: h0cols((_s - 1) % CHUNK, k),
                      h0ap_write(s))

        def l1_step(s):
            lstm_step(s, u1, zl1, c1,
                      lambda k: h1[:, k * BL:(k + 1) * BL],
                      h1[:, :].rearrange("p (k b) -> p k b", k=KT))

        # ---- prologue: L0 chunk 0 ----
        proj_l0(0)
        for s in range(CHUNK):
            l0_step(s)

        # ---- main loop: L0 chunk i (t0=iv), L1 chunk i-1 ----
        with tc.For_i(CHUNK, T, CHUNK) as iv:
            proj_l1()
            proj_l0(iv)
            for s in range(CHUNK):
                l0_step(s)
                l1_step(s)

        # ---- epilogue: L1 chunk 31, FC head ----
        proj_l1()
        for s in range(CHUNK):
            l1_step(s)

        psf = zl0[:, 0:BL]
        for k in range(KT):
            nc.tensor.matmul(psf, lhsT=wf[k][:, :],
                             rhs=h1[:, k * BL:(k + 1) * BL],
                             start=(k == 0), stop=(k == KT - 1))
        oT = work.tile([128, BL], f32, tag="oT")
        nc.scalar.activation(oT[:, :], psf, AF.Identity, bias=bfct[:, 0:1])
        nc.sync.dma_start(out_d[:, :], oT[:, :])

    nc.compile()
    return nc


def _get_compiled():
    if "main" not in _cache:
        _cache["main"] = _build()
    return _cache["main"]


def _ballmat(b, perm, mdt):
    m = np.zeros((128, 128), np.float32)
    m[0:NM, :] = np.asarray(b, np.float32)[perm].reshape(NM, 128)
    return np.ascontiguousarray(m.astype(mdt))


def _onehot(mdt):
    # z bank layout is tg-major: col = tg*(NM*BL) + m*BL + b -> row m hot
    # wherever (col % (NM*BL)) // BL == m
    m = np.zeros((128, BKCOLS), np.float32)
    cols = np.arange(BKCOLS)
    m[(cols % (NM * BL)) // BL, cols] = 1.0
    return np.ascontiguousarray(m.astype(mdt))


def _in_maps(input_seq, W0, U0, b0, W1, U1, b1, Wfc, bfc):
    mdt = _np_mmdt()
    x = np.asarray(input_seq, dtype=np.float32)
    # reorder gate blocks (i,f,g,o) -> (i,f,o,g)
    perm = np.concatenate([np.arange(0, 2 * H),
                           np.arange(3 * H, 4 * H),
                           np.arange(2 * H, 3 * H)])

    def gp(w):
        return np.ascontiguousarray(
            np.asarray(w, np.float32)[..., perm].astype(mdt))

    shared = {
        "w0": gp(W0),
        "u0": gp(U0),
        "w1": gp(W1),
        "u1": gp(U1),
        "wfc": np.ascontiguousarray(np.asarray(Wfc, np.float32).astype(mdt)),
        "b0r": _ballmat(b0, perm, mdt),
        "b1r": _ballmat(b1, perm, mdt),
        "oneh": _onehot(mdt),
        "bfct": np.ascontiguousarray(
            np.asarray(bfc, np.float32).reshape(1, 128).T),
    }
    in_maps = []
    for c in range(NCORES):
        xs = x[c * BL:(c + 1) * BL]                       # [BL, T, I]
        xp = np.ascontiguousarray(xs.transpose(2, 0, 1).astype(mdt))
        m = dict(shared)
        m["xprep"] = xp
        in_maps.append(m)
    return in_maps


def _assemble(res):
    out = np.empty((B, 1, O), np.float32)
    for c in range(NCORES):
        out[c * BL:(c + 1) * BL, 0, :] = res.results[c]["outT"].T
    return out


def _run(nc, inputs):
    from concourse.bass_utils import run_bass_kernel_spmd
    in_maps = _in_maps(**inputs)
    res = run_bass_kernel_spmd(nc, in_maps, list(range(NCORES)))
    return _assemble(res)


def kernel(input_seq, W0, U0, b0, W1, U1, b1, Wfc, bfc):
    nc = _get_compiled()
    return _run(nc, dict(input_seq=input_seq, W0=W0, U0=U0, b0=b0, W1=W1,
                         U1=U1, b1=b1, Wfc=Wfc, bfc=bfc))


# revision 26
# speedup vs baseline: 2.1775x; 1.1959x over previous
"""Two-layer LSTM (linear cell/output activations) + FC head on 8 NeuronCores.

Strategy (data-parallel over batch, per the sharding hint):
  - B=32 split across 8 cores -> B_local=4 per core; weights replicated.
  - State transposed: h^T/c^T are [H on partitions, (k,b) on free]; the
    per-step recurrence is z^T += U^T @ h^T with U stationary, landing as
    [128, 4] column blocks in PSUM.
  - z lives ENTIRELY in PSUM: the per-chunk input projections (x@W0, h0@W1)
    and rank-1 bias matmuls write xw+b into PSUM up front; the per-step
    recurrence matmuls accumulate U@h on top (start=False). No DVE add.
  - PSUM layout: 4 banks per layer, bank = t%4 (parity). The engine reading
    z for step t (ACT) touches bank t%4 while the PE writes bank (t+1)%4 —
    never the same bank, so Tile's bank tracker keeps them parallel.
  - The two layers are software-pipelined ONE CHUNK apart: the loop body
    interleaves L0 step s of chunk i with L1 step s of chunk i-1. Each
    layer's sigmoid+gate-math chain hides under the other layer's matmul
    block. Prologue runs L0 chunk 0; epilogue runs L1 chunk 31 + FC.
  - Per-step chain: ACT copies g to SBUF + sigmoids i,f (early, after their
    m-blocks) and o; DVE does 4 small SBUF-only tensor_tensor ops. PE order
    per step: g-blocks, i, f, o so the sigmoid can start at ~75% of the
    matmul block.
"""

import os
import numpy as np
from contextlib import ExitStack

os.environ.setdefault("MYCRO_LOCAL_CACHE", "1")

B, T, I, H, O = 32, 2048, 128, 256, 128
NCORES = 8
BL = B // NCORES          # 4 batch elements per core
CHUNK = 64                # timesteps per loop body
G4 = 4 * H                # 1024 gate columns
NM = G4 // 128            # 8 gate m-blocks of 128
KT = H // 128             # 2 contraction tiles
NP = 4                    # PSUM bank parity groups
TG = CHUNK // NP          # 16 step-groups per parity
BKCOLS = TG * NM * BL     # 512 fp32 cols per bank

MM_BF16 = True

_cache = {}


def _np_mmdt():
    if MM_BF16:
        import ml_dtypes
        return ml_dtypes.bfloat16
    return np.float32


def _build(tiny=False):
    import concourse.bacc as bacc
    import concourse.bass as bass
    import concourse.tile as tile
    import concourse.mybir as mybir

    f32 = mybir.dt.float32
    mdt = mybir.dt.bfloat16 if MM_BF16 else f32
    AF = mybir.ActivationFunctionType
    ALU = mybir.AluOpType

    nc = bacc.Bacc("TRN2", target_bir_lowering=False, debug=False,
                   num_devices=NCORES)

    xprep_d = nc.declare_dram_parameter("xprep", [I, BL, T], mdt, isOutput=False)
    w0_d = nc.declare_dram_parameter("w0", [I, G4], mdt, isOutput=False)
    u0_d = nc.declare_dram_parameter("u0", [H, G4], mdt, isOutput=False)
    w1_d = nc.declare_dram_parameter("w1", [H, G4], mdt, isOutput=False)
    u1_d = nc.declare_dram_parameter("u1", [H, G4], mdt, isOutput=False)
    wfc_d = nc.declare_dram_parameter("wfc", [H, O], mdt, isOutput=False)
    b0r_d = nc.declare_dram_parameter("b0r", [128, 128], mdt, isOutput=False)
    b1r_d = nc.declare_dram_parameter("b1r", [128, 128], mdt, isOutput=False)
    oneh_d = nc.declare_dram_parameter("oneh", [128, BKCOLS], mdt,
                                       isOutput=False)
    bfct_d = nc.declare_dram_parameter("bfct", [128, 1], f32, isOutput=False)
    out_d = nc.declare_dram_parameter("outT", [O, BL], f32, isOutput=True)

    with tile.TileContext(nc) as tc, ExitStack() as ctx:
        if tiny:
            pool = ctx.enter_context(tc.tile_pool(name="tp", bufs=1))
            t1 = pool.tile([128, BL], mdt, tag="t1")
            t2 = pool.tile([128, BL], f32, tag="t2")
            nc.sync.dma_start(t1[:, :], xprep_d[:, :, 0])
            nc.vector.tensor_copy(t2[:, :], t1[:, :])
            nc.sync.dma_start(out_d[:, :], t2[:, :])
            nc.compile()
            return nc

        const = ctx.enter_context(tc.tile_pool(name="const", bufs=1))
        work = ctx.enter_context(tc.tile_pool(name="work", bufs=4))
        # single-slot pool for ig: the WAR on the shared slot forces the
        # scheduler to finish stream A's c-update before starting stream
        # B's ig, keeping the DVE FIFO round-robin between the two layers.
        igp = ctx.enter_context(tc.tile_pool(name="igp", bufs=1))
        psum = ctx.enter_context(tc.tile_pool(name="psum", bufs=1, space="PSUM"))

        # ---- persistent SBUF residents ----
        # xall col = b*T + t
        xall = const.tile([128, BL * T], mdt, tag="xall")
        w0 = const.tile([128, G4], mdt, tag="w0")
        u0 = [const.tile([128, G4], mdt, tag=f"u0_{k}", name=f"u0_{k}")
              for k in range(KT)]
        w1 = [const.tile([128, G4], mdt, tag=f"w1_{k}", name=f"w1_{k}")
              for k in range(KT)]
        u1 = [const.tile([128, G4], mdt, tag=f"u1_{k}", name=f"u1_{k}")
              for k in range(KT)]
        wf = [const.tile([128, O], mdt, tag=f"wf_{k}", name=f"wf_{k}")
              for k in range(KT)]
        # bias matrices for the rank-1 bias matmuls: row r (partition r) =
        # bias of m-block r; block-one-hot rhs maps row m to col block m,
        # so ONE matmul adds every m-block's bias to a whole PSUM bank.
        ball0 = const.tile([128, 128], mdt, tag="ball0")
        ball1 = const.tile([128, 128], mdt, tag="ball1")
        onehot = const.tile([128, BKCOLS], mdt, tag="onehot")
        bfct = const.tile([128, 1], f32, tag="bfct")
        # h0 stream for one chunk, parity-major: col = k*256 + (s%4)*64
        #   + (s//4)*4 + b
        h0t = const.tile([128, KT * CHUNK * BL], mdt, tag="h0t")
        # recurrent state, col = k*BL + b
        c0 = const.tile([128, KT * BL], f32, tag="c0")
        c1 = const.tile([128, KT * BL], f32, tag="c1")
        h1 = const.tile([128, KT * BL], mdt, tag="h1")

        # ---- PSUM: z for each layer, 4 banks, bank = step parity ----
        # col = (s%4)*BKCOLS + m*TG*BL + (s//4)*BL + b
        zl0 = psum.tile([128, NP * BKCOLS], f32, tag="zl0")
        zl1 = psum.tile([128, NP * BKCOLS], f32, tag="zl1")

        nc.sync.dma_start(xall[:, :].rearrange("p (b t) -> p b t", b=BL),
                          xprep_d[:, :, :])
        nc.sync.dma_start(w0[:, :], w0_d[:, :])
        for k in range(KT):
            sl = slice(k * 128, (k + 1) * 128)
            nc.sync.dma_start(u0[k][:, :], u0_d[sl, :])
            nc.sync.dma_start(w1[k][:, :], w1_d[sl, :])
            nc.sync.dma_start(u1[k][:, :], u1_d[sl, :])
            nc.sync.dma_start(wf[k][:, :], wfc_d[sl, :])
        nc.sync.dma_start(ball0[:, :], b0r_d[:, :])
        nc.sync.dma_start(ball1[:, :], b1r_d[:, :])
        nc.sync.dma_start(bfct[:, :], bfct_d[:, :])
        nc.sync.dma_start(onehot[:, :], oneh_d[:, :])
        nc.vector.memset(h0t[:, :], 0.0)
        nc.vector.memset(c0[:, :], 0.0)
        nc.vector.memset(c1[:, :], 0.0)
        nc.vector.memset(h1[:, :], 0.0)

        # z bank layout is tg-major: col = pr*BKCOLS + tg*(NM*BL) + m*BL + b
        # so one step's z is 32 CONTIGUOUS fp32 (i,f,o = 24, g = 8).
        # h0 layout: col = k*CHUNK*BL + pr*TG*BL + tg*BL + b
        h0v = h0t[:, :].rearrange("p (k pr tg b) -> p k pr tg b",
                                  k=KT, pr=NP, tg=TG)

        SW = NM * BL          # 32 z cols per step

        def zstep(zl, s):
            """offset of step s's z block (32 cols)."""
            return (s % NP) * BKCOLS + (s // NP) * SW

        def zcols(zl, s, m):
            """z AP for step s, m-block m: [128, BL]."""
            base = zstep(zl, s) + m * BL
            return zl[:, base:base + BL]

        def h0cols(s, k):
            return h0v[:, k, s % NP, s // NP]

        def h0ap_write(s):
            """[128, KT, BL] write AP for step s's h in h0t."""
            return h0v[:, :, s % NP, s // NP]

        # m-block order: g (6,7) first, then i (0,1), f (2,3), o (4,5)
        M_ORDER = [6, 7, 0, 1, 2, 3, 4, 5]

        def lstm_step(s, uw, zl, cst, h_rhs_fn, h_out_ap):
            S = KT * BL
            for m in M_ORDER:
                for k in range(KT):
                    nc.tensor.matmul(zcols(zl, s, m),
                                     lhsT=uw[k][:, m * 128:(m + 1) * 128],
                                     rhs=h_rhs_fn(k),
                                     start=False, stop=(k == KT - 1))
            zb = zstep(zl, s)
            # one sigmoid covers i,f,o; g is consumed straight from PSUM
            sg = work.tile([128, 3 * S], f32, tag="sg")
            nc.scalar.activation(sg[:, :], zl[:, zb:zb + 3 * S], AF.Sigmoid)
            # DVE: c = f*c + i*g ; h = o*c
            ig = igp.tile([128, S], f32, tag="ig")
            nc.vector.tensor_tensor(cst[:, :], sg[:, S:2 * S], cst[:, :],
                                    ALU.mult)
            nc.vector.tensor_tensor(ig[:, :], sg[:, 0:S],
                                    zl[:, zb + 3 * S:zb + 4 * S], ALU.mult)
            nc.vector.tensor_tensor(cst[:, :], cst[:, :], ig[:, :], ALU.add)
            nc.vector.tensor_tensor(
                h_out_ap,
                sg[:, 2 * S:3 * S].rearrange("p (k b) -> p k b", k=KT),
                cst[:, :].rearrange("p (k b) -> p k b", k=KT), ALU.mult)

        def proj_bias(zl, ball):
            """z += bias: one N=512 matmul per bank adds every m-block's
            bias (ball row m) to its col block via the block-one-hot rhs."""
            for p in range(NP):
                nc.tensor.matmul(
                    zl[:, p * BKCOLS:(p + 1) * BKCOLS],
                    lhsT=ball[:, :], rhs=onehot[:, :],
                    start=False, stop=False)

        # proj-output view [p, pr, tg, m, b] (strided per m-block)
        zp0 = zl0[:, :].rearrange("p (pr tg m b) -> p pr tg m b",
                                  pr=NP, tg=TG, m=NM)
        zp1 = zl1[:, :].rearrange("p (pr tg m b) -> p pr tg m b",
                                  pr=NP, tg=TG, m=NM)

        def proj_l0(iv):
            """xw0 for chunk at t0=iv into zl0 (start=True clears banks)."""
            xq = work.tile([128, BL * CHUNK], mdt, tag="xq")
            # xq col = (t%4)*64 + (t//4)*4 + b  <- xall[b*T + iv + p + 4*tg]
            src = xall[:, :].rearrange("p (b t) -> p b t", b=BL)
            nc.vector.tensor_copy(
                xq[:, :].rearrange("p (pr tg b) -> p pr tg b", pr=NP, tg=TG),
                src[:, :, bass.ds(iv, CHUNK)].rearrange(
                    "p b (tg pr) -> p pr tg b", pr=NP),
            )
            for m in range(NM):
                for p in range(NP):
                    nc.tensor.matmul(
                        zp0[:, p, :, m],
                        lhsT=w0[:, m * 128:(m + 1) * 128],
                        rhs=xq[:, p * TG * BL:(p + 1) * TG * BL],
                        start=(m == 0), stop=False)
            proj_bias(zl0, ball0)

        def proj_l1():
            """xw1 = W1 @ h0(prev chunk) into zl1."""
            for m in range(NM):
                for p in range(NP):
                    for k in range(KT):
                        nc.tensor.matmul(
                            zp1[:, p, :, m],
                            lhsT=w1[k][:, m * 128:(m + 1) * 128],
                            rhs=h0t[:, k * CHUNK * BL + p * TG * BL:
                                    k * CHUNK * BL + (p + 1) * TG * BL],
                            start=(m == 0 and k == 0), stop=False)
            proj_bias(zl1, ball1)

        def l0_step(s):
            lstm_step(s, u0, zl0, c0,
                      lambda k, _s=s: h0cols((_s - 1) % CHUNK, k),
                      h0ap_write(s))

        def l1_step(s):
            lstm_step(s, u1, zl1, c1,
                      lambda k: h1[:, k * BL:(k + 1) * BL],
                      h1[:, :].rearrange("p (k b) -> p k b", k=KT))

        # ---- prologue: L0 chunk 0 ----
        proj_l0(0)
        for s in range(CHUNK):
            l0_step(s)

        # ---- main loop: L0 chunk i (t0=iv), L1 chunk i-1 ----
        from concourse.engine_type import EngineType
        with tc.For_i(CHUNK, T, CHUNK,
                      hint_engines=(EngineType.PE, EngineType.DVE,
                                    EngineType.Activation)) as iv:
            proj_l1()
            proj_l0(iv)
            for s in range(CHUNK):
                l0_step(s)
                l1_step(s)

        # ---- epilogue: L1 chunk 31, FC head ----
        proj_l1()
        for s in range(CHUNK):
            l1_step(s)

        psf = zl0[:, 0:BL]
        for k in range(KT):
            nc.tensor.matmul(psf, lhsT=wf[k][:, :],
                             rhs=h1[:, k * BL:(k + 1) * BL],
                             start=(k == 0), stop=(k == KT - 1))
        oT = work.tile([128, BL], f32, tag="oT")
        nc.scalar.activation(oT[:, :], psf, AF.Identity, bias=bfct[:, 0:1])
        nc.sync.dma_start(out_d[:, :], oT[:, :])

    nc.compile()
    return nc


def _get_compiled():
    if "main" not in _cache:
        _cache["main"] = _build()
    return _cache["main"]


def _ballmat(b, perm, mdt):
    m = np.zeros((128, 128), np.float32)
    m[0:NM, :] = np.asarray(b, np.float32)[perm].reshape(NM, 128)
    return np.ascontiguousarray(m.astype(mdt))


def _onehot(mdt):
    # z bank layout is tg-major: col = tg*(NM*BL) + m*BL + b -> row m hot
    # wherever (col % (NM*BL)) // BL == m
    m = np.zeros((128, BKCOLS), np.float32)
    cols = np.arange(BKCOLS)
    m[(cols % (NM * BL)) // BL, cols] = 1.0
    return np.ascontiguousarray(m.astype(mdt))


def _in_maps(input_seq, W0, U0, b0, W1, U1, b1, Wfc, bfc):
    mdt = _np_mmdt()
    x = np.asarray(input_seq, dtype=np.float32)
    # reorder gate blocks (i,f,g,o) -> (i,f,o,g)
    perm = np.concatenate([np.arange(0, 2 * H),
                           np.arange(3 * H, 4 * H),
                           np.arange(2 * H, 3 * H)])

    def gp(w):
        return np.ascontiguousarray(
            np.asarray(w, np.float32)[..., perm].astype(mdt))

    shared = {
        "w0": gp(W0),
        "u0": gp(U0),
        "w1": gp(W1),
        "u1": gp(U1),
        "wfc": np.ascontiguousarray(np.asarray(Wfc, np.float32).astype(mdt)),
        "b0r": _ballmat(b0, perm, mdt),
        "b1r": _ballmat(b1, perm, mdt),
        "oneh": _onehot(mdt),
        "bfct": np.ascontiguousarray(
            np.asarray(bfc, np.float32).reshape(1, 128).T),
    }
    in_maps = []
    for c in range(NCORES):
        xs = x[c * BL:(c + 1) * BL]                       # [BL, T, I]
        xp = np.ascontiguousarray(xs.transpose(2, 0, 1).astype(mdt))
        m = dict(shared)
        m["xprep"] = xp
        in_maps.append(m)
    return in_maps


def _assemble(res):
    out = np.empty((B, 1, O), np.float32)
    for c in range(NCORES):
        out[c * BL:(c + 1) * BL, 0, :] = res.results[c]["outT"].T
    return out


def _run(nc, inputs):
    from concourse.bass_utils import run_bass_kernel_spmd
    in_maps = _in_maps(**inputs)
    res = run_bass_kernel_spmd(nc, in_maps, list(range(NCORES)))
    return _assemble(res)


def kernel(input_seq, W0, U0, b0, W1, U1, b1, Wfc, bfc):
    nc = _get_compiled()
    return _run(nc, dict(input_seq=input_seq, W0=W0, U0=U0, b0=b0, W1=W1,
                         U1=U1, b1=b1, Wfc=Wfc, bfc=bfc))
